# revision 29
# baseline (speedup 1.0000x reference)
"""Trainium2 Bass kernel for ChunkGatedAttentionUnit.

Sharding: 2 batch groups x 4-way tensor parallel on expanded dim D.
Core c handles batch b=c//4, D-slice j=c%4 (DL=512 columns).

Per core:
  - projections of its q^T/k^T/k/v/g slices from X^T (bf16 matmuls, fp32 psum)
  - AllGather of q^T, k^T, k(natural) within the 4-core batch group
  - per-chunk: causal softmax attention (intra) + linear-attention state
    scan (inter) + gating + out-projection partial
  - ReduceScatter of the out-projection partials; host concatenates.

The state scan keeps an exact fp32 master in SBUF, snapshots it to bf16
every 2 chunks for the cross matmuls, and corrects the stale odd chunks
with an exact rank-128 pair term (q_i k_{i-1}^T) v_{i-1}.
"""

import sys
import math

sys.path.insert(0, "/opt/trn_rl_repo")

import numpy as np
import concourse.bass as bass
from concourse import mybir
from concourse import masks
from concourse.tile import TileContext
from concourse.bass_utils import run_bass_kernel_spmd

FP32 = mybir.dt.float32
FP16 = mybir.dt.float16
BF16 = mybir.dt.bfloat16
ACT_COPY = mybir.ActivationFunctionType.Copy
ACT_EXP = mybir.ActivationFunctionType.Exp
ACT_SIGMOID = mybir.ActivationFunctionType.Sigmoid

B, S, H, D = 2, 4096, 1024, 2048
CS = 128
NCORE = 8
GROUP = 4
SW = 512  # state free-dim stride per d-tile


def split_excess_waits(nc, limit=1):
    """This walrus build rejects instructions with >limit sync waits; move
    excess waits onto standalone event-semaphore instructions just before."""
    n = 0
    for f in nc.m.functions:
        for bb in f.blocks:
            new_insts = []
            for inst in bb.instructions:
                si = inst.sync_info
                if si is not None and si.on_wait and len(si.on_wait) > limit:
                    waits = list(si.on_wait)
                    excess, keep = waits[:-limit], waits[-limit:]
                    for j in range(0, len(excess), limit):
                        n += 1
                        es = mybir.InstEventSemaphore(
                            name=f"{inst.name}_wsplit{n}",
                            ins=[],
                            outs=[],
                            sync_info=mybir.SyncInfo(
                                on_wait=excess[j : j + limit], on_update=[]
                            ),
                        )
                        es.engine = inst.engine
                        new_insts.append(es)
                    si.on_wait = keep
                new_insts.append(inst)
            bb.instructions = new_insts
    return n


def build(S=S, H=H, D=D, cs=CS, ncore=NCORE, group=GROUP, split_waits=True, nspl=4,
          with_bias=True):
    """Emit the SPMD Tile program. Returns nc."""
    DL = D // group
    nH = H // 128
    nC = S // cs
    nDT = D // 128
    nDL = DL // 128
    nSS = S // 512
    NSPL = min(nspl, nSS)    # collective pipeline blocks over the s axis
    Sn = S // NSPL           # rows per block
    nSSb = nSS // NSPL       # 512-slices per block
    cpb = nC // NSPL         # chunks per block
    Sg = Sn // group         # (AG-block rows per rank, kept for reference)
    RSPL = min(2, NSPL)      # reduce-scatter blocks (floor-dominated)
    Sn2 = S // RSPL          # rows per RS block
    cpr = nC // RSPL         # chunks per RS block
    Sg2 = Sn2 // group       # RS output rows per rank per block
    scale = 1.0 / math.sqrt(D)
    groups = [list(range(g * group, (g + 1) * group))
              for g in range(ncore // group)]

    nc = bass.Bass("TRN2", target_bir_lowering=False, debug=False,
                   num_devices=ncore)

    # ---- I/O (X^T pre-transposed on host) ----
    xt_in = nc.dram_tensor("xt", [H, S], BF16, kind="ExternalInput")
    wq_in = nc.dram_tensor("wq", [H, DL], BF16, kind="ExternalInput")
    wk_in = nc.dram_tensor("wk", [H, DL], BF16, kind="ExternalInput")
    wv_in = nc.dram_tensor("wv", [H, DL], BF16, kind="ExternalInput")
    wg_in = nc.dram_tensor("wg", [H, DL], BF16, kind="ExternalInput")
    wo_in = nc.dram_tensor("wo", [DL, H], BF16, kind="ExternalInput")
    bq_in = nc.dram_tensor("bq", [DL], FP32, kind="ExternalInput")
    bk_in = nc.dram_tensor("bk", [DL], FP32, kind="ExternalInput")
    bv_in = nc.dram_tensor("bv", [DL], FP32, kind="ExternalInput")
    bg_in = nc.dram_tensor("bg", [DL], FP32, kind="ExternalInput")
    bo_in = nc.dram_tensor("bo", [H], FP32, kind="ExternalInput")
    y_out = nc.dram_tensor("y_red", [S // group, H], FP32,
                           kind="ExternalOutput")

    # ---- internal DRAM (s-blocked for collective pipelining) ----
    # q^T and k^T share one gathered tensor: index 0 = q^T, 1 = k^T
    qk_my = nc.dram_tensor("qk_my", [NSPL, 2, DL, Sn], BF16)
    g_my = nc.dram_tensor("g_my", [S, DL], FP32)
    qk_ag = nc.dram_tensor("qk_ag", [NSPL, group, 2, DL, Sn], BF16)
    part_out = nc.dram_tensor("part_out", [S, H], FP16)
    rs_out = nc.dram_tensor("rs_out", [RSPL, Sg2, H], FP16)

    with TileContext(nc) as tc:
        from contextlib import ExitStack
        with ExitStack() as stack:
            const_pool = stack.enter_context(tc.tile_pool(name="const", bufs=1))
            wo_pool = stack.enter_context(tc.tile_pool(name="wo", bufs=1))
            vres_pool = stack.enter_context(tc.tile_pool(name="vres", bufs=1))
            v_sb = vres_pool.tile([128, (S // 128) * DL], BF16, tag="vsb")

            ident = const_pool.tile([128, 128], BF16, tag="ident")
            masks.make_identity(nc, ident[:])
            causal = const_pool.tile([128, 128], FP32, tag="causal")
            masks.make_causal_mask(nc, causal[:], mask_val=-30000.0)
            ones_row = const_pool.tile([1, 512], BF16, tag="ones")
            nc.gpsimd.memset(ones_row[:], 1.0)

            bias_sb = {}
            for name, dram, width in (("bq", bq_in, DL), ("bk", bk_in, DL),
                                      ("bv", bv_in, DL), ("bg", bg_in, DL),
                                      ("bo", bo_in, H)):
                bf = const_pool.tile([1, width], FP32, tag=name + "f")
                nc.sync.dma_start(out=bf[:], in_=dram[None, :])
                bb16 = const_pool.tile([1, width], BF16, tag=name)
                nc.scalar.activation(bb16[:], bf[:], ACT_COPY)
                bias_sb[name] = bb16

            # Wo resident through attention: [p=dl%128, (dl_tile, H)]
            wo_sb = wo_pool.tile([128, nDL * H], BF16, tag="wo")
            for t in range(nDL):
                nc.sync.dma_start(out=wo_sb[:, t * H:(t + 1) * H],
                                  in_=wo_in[t * 128:(t + 1) * 128, :])

            # ---------- phase 1: projections ----------
            with tc.tile_pool(name="xt", bufs=1) as xt_pool, \
                 tc.tile_pool(name="wsb", bufs=1) as wsb_pool, \
                 tc.tile_pool(name="pj_psum", bufs=4, space="PSUM") as pj_psum, \
                 tc.tile_pool(name="pj_stage", bufs=4) as pj_stage:

                # X^T -> sbuf bf16: [p=h%128, (ht, S)]
                xt_sb = xt_pool.tile([128, nH * S], BF16, tag="xt")
                for ht in range(nH):
                    nc.sync.dma_start(out=xt_sb[:, ht * S:(ht + 1) * S],
                                      in_=xt_in[ht * 128:(ht + 1) * 128, :])

                # weights -> sbuf bf16: [p=h%128, (ht, DL)]
                w_sb = {}
                for name, dram in (("wq", wq_in), ("wk", wk_in),
                                   ("wv", wv_in), ("wg", wg_in)):
                    wt = wsb_pool.tile([128, nH * DL], BF16, tag=name)
                    for ht in range(nH):
                        nc.sync.dma_start(
                            out=wt[:, ht * DL:(ht + 1) * DL],
                            in_=dram[ht * 128:(ht + 1) * 128, :])
                    w_sb[name] = wt

                # blocked over the s axis so each block's all-gathers start
                # as soon as that block's projections land
                for blk in range(NSPL):
                    # q^T / k^T slices: out [128 d rows, 512 s cols]
                    for qk, bname, wname in ((0, "bq", "wq"), (1, "bk", "wk")):
                        wt = w_sb[wname]
                        for dd in range(nDL):
                            for ssl in range(nSSb):
                                ss = blk * nSSb + ssl
                                ps = pj_psum.tile([128, 512], FP32, tag="ps")
                                for ht in range(nH):
                                    nc.tensor.matmul(
                                        ps[:],
                                        wt[:, ht * DL + dd * 128:
                                           ht * DL + dd * 128 + 128],
                                        xt_sb[:, ht * S + ss * 512:
                                              ht * S + ss * 512 + 512],
                                        start=(ht == 0),
                                        stop=(not with_bias
                                              and ht == nH - 1))
                                # += bias[d] * ones[s]
                                if with_bias:
                                    nc.tensor.matmul(
                                        ps[:],
                                        bias_sb[bname][0:1, dd * 128:
                                                       dd * 128 + 128],
                                        ones_row[0:1, 0:512],
                                        start=False, stop=True)
                                st = pj_stage.tile([128, 512], BF16, tag="st")
                                nc.scalar.activation(st[:], ps[:], ACT_COPY)
                                nc.sync.dma_start(
                                    out=qk_my[blk, qk,
                                              dd * 128:(dd + 1) * 128,
                                              ssl * 512:ssl * 512 + 512],
                                    in_=st[:])

                    # v / g slices: out [128 s rows, 512 d cols]
                    for bname, wname, dest, act in (
                            ("bv", "wv", None, "v"),
                            ("bg", "wg", g_my, "sig")):
                        wt = w_sb[wname]
                        for stl in range(Sn // 128):
                            st_i = blk * (Sn // 128) + stl
                            for dd in range(0, DL, 512):
                                dw = min(512, DL - dd)
                                ps = pj_psum.tile([128, 512], FP32, tag="ps")
                                for ht in range(nH):
                                    nc.tensor.matmul(
                                        ps[:, 0:dw],
                                        xt_sb[:, ht * S + st_i * 128:
                                              ht * S + st_i * 128 + 128],
                                        wt[:, ht * DL + dd:
                                           ht * DL + dd + dw],
                                        start=(ht == 0),
                                        stop=(not with_bias
                                              and ht == nH - 1))
                                # += ones[s] * bias[d]
                                if with_bias:
                                    nc.tensor.matmul(
                                        ps[:, 0:dw], ones_row[0:1, 0:128],
                                        bias_sb[bname][0:1, dd:dd + dw],
                                        start=False, stop=True)
                                if act == "v":
                                    # straight into the resident SBUF tile
                                    nc.scalar.activation(
                                        v_sb[:, st_i * DL + dd:
                                             st_i * DL + dd + dw],
                                        ps[:, 0:dw], ACT_COPY)
                                elif act == "sig":
                                    stg = pj_stage.tile([128, 512], FP32,
                                                        tag="stgf")
                                    nc.scalar.activation(
                                        stg[:, 0:dw], ps[:, 0:dw],
                                        ACT_SIGMOID)
                                    nc.sync.dma_start(
                                        out=dest[st_i * 128:(st_i + 1) * 128,
                                                 dd:dd + dw],
                                        in_=stg[:, 0:dw])

                    # one all-gather per block covers q^T and k^T
                    nc.gpsimd.collective_compute(
                        "AllGather", mybir.AluOpType.bypass,
                        ins=[qk_my[blk]], outs=[qk_ag[blk]],
                        replica_groups=groups)

            with tc.tile_pool(name="state", bufs=1) as state_pool, \
                 tc.tile_pool(name="snap", bufs=2) as snap_pool, \
                 tc.tile_pool(name="chin", bufs=4) as chin_pool, \
                 tc.tile_pool(name="sm", bufs=3) as sm_pool, \
                 tc.tile_pool(name="ysb", bufs=3) as ysb_pool, \
                 tc.tile_pool(name="ostage", bufs=3) as ostage_pool, \
                 tc.tile_pool(name="sc_ps", bufs=1, space="PSUM") as sc_ps_pool, \
                 tc.tile_pool(name="y_ps", bufs=2, space="PSUM") as y_ps_pool, \
                 tc.tile_pool(name="d_ps", bufs=2, space="PSUM") as d_ps_pool, \
                 tc.tile_pool(name="t_ps", bufs=1, space="PSUM") as t_ps_pool, \
                 tc.tile_pool(name="o_ps", bufs=1, space="PSUM") as o_ps_pool:

                state = state_pool.tile([128, nDT * SW], FP32, tag="state")
                snap = None
                prev = None

                for i in range(nC):
                    blk, il = i // cpb, i % cpb
                    s0 = il * cs
                    qTc = chin_pool.tile([128, nDT * 128], BF16, tag="qTc")
                    kTc = chin_pool.tile([128, nDT * 128], BF16, tag="kTc")
                    for r in range(group):
                        for qk, dst in ((0, qTc), (1, kTc)):
                            nc.sync.dma_start(
                                out=dst[:, r * nDL * 128:
                                        (r + 1) * nDL * 128].rearrange(
                                    "p (t s) -> p t s", s=cs),
                                in_=qk_ag[blk, r, qk].rearrange(
                                    "(t p) s -> p t s", p=128)[:, :,
                                                               s0:s0 + cs])
                    vc = v_sb[:, i * DL:(i + 1) * DL]
                    gc = chin_pool.tile([128, DL], FP32, tag="gc")
                    nc.sync.dma_start(out=gc[:],
                                      in_=g_my[i * cs:(i + 1) * cs, :])

                    # --- y psum: cross (from snapshot) first, local last ---
                    y_ps = y_ps_pool.tile([128, DL], FP32, tag="yps")
                    first = True
                    if i > 1:
                        for t in range(nDT):
                            nc.tensor.matmul(
                                y_ps[:], qTc[:, t * 128:(t + 1) * 128],
                                snap[:, t * SW:t * SW + DL],
                                start=first, stop=False)
                            first = False
                    if i % 2 == 1:
                        # exact pair term (q_i k_{i-1}^T) v_{i-1}
                        pqTc, pkTc, pvc = prev
                        tij_ps = sc_ps_pool.tile([128, 128], FP32, tag="sc")
                        for t in range(nDT):
                            nc.tensor.matmul(
                                tij_ps[:], pkTc[:, t * 128:(t + 1) * 128],
                                qTc[:, t * 128:(t + 1) * 128],
                                start=(t == 0), stop=(t == nDT - 1))
                        tij = sm_pool.tile([128, 128], BF16, tag="tij")
                        nc.vector.tensor_copy(tij[:], tij_ps[:])
                        nc.tensor.matmul(y_ps[:], tij[:], pvc[:],
                                         start=first, stop=False)
                        first = False

                    # --- intra-chunk causal softmax ---
                    sc = sc_ps_pool.tile([128, 128], FP32, tag="sc")
                    for t in range(nDT):
                        nc.tensor.matmul(sc[:], qTc[:, t * 128:(t + 1) * 128],
                                         kTc[:, t * 128:(t + 1) * 128],
                                         start=(t == 0), stop=(t == nDT - 1))
                    masked = sm_pool.tile([128, 128], FP32, tag="masked")
                    nc.vector.tensor_add(masked[:], sc[:], causal[:])
                    mx = sm_pool.tile([128, 1], FP32, tag="mx")
                    nc.vector.reduce_max(mx[:], masked[:],
                                         axis=mybir.AxisListType.X)
                    nbias = sm_pool.tile([128, 1], FP32, tag="nbias")
                    nc.scalar.activation(nbias[:], mx[:], ACT_COPY,
                                         scale=-scale)
                    probs = sm_pool.tile([128, 128], BF16, tag="probs")
                    denom = sm_pool.tile([128, 1], FP32, tag="denom")
                    nc.scalar.activation(probs[:], masked[:], ACT_EXP,
                                         bias=nbias[:], scale=scale,
                                         accum_out=denom[:])
                    rden = sm_pool.tile([128, 1], FP32, tag="rden")
                    nc.vector.reciprocal(rden[:], denom[:])
                    probsn = sm_pool.tile([128, 128], BF16, tag="probsn")
                    nc.vector.tensor_scalar_mul(probsn[:], probs[:], rden[:])
                    pt_ps = t_ps_pool.tile([128, 512], BF16, tag="tps")
                    nc.tensor.transpose(pt_ps[:, 0:128], probsn[:], ident[:])
                    pt = sm_pool.tile([128, 128], BF16, tag="pt")
                    nc.vector.tensor_copy(pt[:], pt_ps[:, 0:128])

                    # local lands last in the y psum group
                    nc.tensor.matmul(y_ps[:], pt[:], vc[:], start=first,
                                     stop=True)

                    # --- gate + transpose y ---
                    y_sb = ysb_pool.tile([128, DL], BF16, tag="ysb")
                    nc.vector.tensor_mul(y_sb[:], y_ps[:], gc[:])
                    yt_ps = t_ps_pool.tile([128, 512], BF16, tag="tps")
                    for c4 in range(nDL):
                        nc.tensor.transpose(
                            yt_ps[:, c4 * 128:(c4 + 1) * 128],
                            y_sb[:, c4 * 128:(c4 + 1) * 128], ident[:])
                    yt = ysb_pool.tile([128, DL], BF16, tag="yt")
                    nc.vector.tensor_copy(yt[:], yt_ps[:, 0:DL])

                    # --- state update (fp32 master in SBUF) ---
                    # k natural tiles come from PE-transposing kTc in place
                    for gi in range(nDT // 4):
                        knt_ps = t_ps_pool.tile([128, 512], BF16, tag="knt")
                        for u in range(4):
                            t = gi * 4 + u
                            nc.tensor.transpose(
                                knt_ps[:, u * 128:(u + 1) * 128],
                                kTc[:, t * 128:(t + 1) * 128], ident[:])
                        knt = ysb_pool.tile([128, 512], BF16, tag="knt_sb")
                        nc.vector.tensor_copy(knt[:], knt_ps[:])
                        for u in range(4):
                            t = gi * 4 + u
                            dps = d_ps_pool.tile([128, 512], FP32, tag="dps")
                            nc.tensor.matmul(
                                dps[:, 0:DL], knt[:, u * 128:(u + 1) * 128],
                                vc[:], start=True, stop=True)
                            if i == 0:
                                nc.vector.tensor_copy(
                                    state[:, t * SW:t * SW + DL],
                                    dps[:, 0:DL])
                            else:
                                nc.vector.tensor_add(
                                    state[:, t * SW:t * SW + DL],
                                    state[:, t * SW:t * SW + DL],
                                    dps[:, 0:DL])
                    if i % 2 == 1 and i + 1 < nC:
                        snap = snap_pool.tile([128, nDT * SW], BF16,
                                              tag="snap")
                        if DL == SW:
                            w = nDT * SW // 4
                            for q4 in range(4):
                                nc.scalar.activation(
                                    snap[:, q4 * w:(q4 + 1) * w],
                                    state[:, q4 * w:(q4 + 1) * w], ACT_COPY)
                        else:
                            for t in range(nDT):
                                nc.scalar.activation(
                                    snap[:, t * SW:t * SW + DL],
                                    state[:, t * SW:t * SW + DL], ACT_COPY)

                    # --- out projection partial (+ bo/group) ---
                    o_sb = ostage_pool.tile([128, H], FP16, tag="osb")
                    for hh in range(H // 512):
                        o_ps = o_ps_pool.tile([128, 512], FP32, tag="ops")
                        for t in range(nDL):
                            nc.tensor.matmul(
                                o_ps[:],
                                yt[:, t * 128:(t + 1) * 128],
                                wo_sb[:, t * H + hh * 512:
                                      t * H + hh * 512 + 512],
                                start=(t == 0),
                                stop=(not with_bias and t == nDL - 1))
                        if with_bias:
                            nc.tensor.matmul(
                                o_ps[:],
                                ones_row[0:1, 0:128],
                                bias_sb["bo"][0:1, hh * 512:hh * 512 + 512],
                                start=False, stop=True)
                        nc.scalar.activation(
                            o_sb[:, hh * 512:(hh + 1) * 512], o_ps[:],
                            ACT_COPY)
                    nc.sync.dma_start(out=part_out[i * cs:(i + 1) * cs, :],
                                      in_=o_sb[:])

                    # reduce-scatter this s block as soon as it's complete
                    if (i + 1) % cpr == 0:
                        r = i // cpr
                        nc.gpsimd.collective_compute(
                            "ReduceScatter", mybir.AluOpType.add,
                            ins=[part_out[r * Sn2:(r + 1) * Sn2, :]],
                            outs=[rs_out[r]], replica_groups=groups)
                        for rr in range(Sg2 // 128):
                            ycvt = ostage_pool.tile([128, H], FP16,
                                                    tag="ycvt")
                            nc.sync.dma_start(
                                out=ycvt[:],
                                in_=rs_out[r, rr * 128:(rr + 1) * 128, :])
                            yf32 = ostage_pool.tile([128, H], FP32,
                                                    tag="yf32")
                            nc.scalar.activation(yf32[:], ycvt[:], ACT_COPY)
                            nc.sync.dma_start(
                                out=y_out[r * Sg2 + rr * 128:
                                          r * Sg2 + (rr + 1) * 128, :],
                                in_=yf32[:])

                    prev = (qTc, kTc, vc)

    if split_waits:
        split_excess_waits(nc)
    return nc


def _prep_inputs(hidden_states, Wq, bq, Wk, bk, Wv, bv, Wg, bg, Wo, bo,
                 ncore=NCORE, group=GROUP):
    import ml_dtypes
    bf16 = ml_dtypes.bfloat16
    D_ = Wq.shape[1]
    DL = D_ // group
    hidden_states = np.asarray(hidden_states, np.float32)
    in_maps = []
    for c in range(ncore):
        b, j = c // group, c % group
        sl = slice(j * DL, (j + 1) * DL)
        in_maps.append({
            "xt": np.ascontiguousarray(hidden_states[b].T).astype(bf16),
            "wq": np.ascontiguousarray(
                np.asarray(Wq, np.float32)[:, sl]).astype(bf16),
            "wk": np.ascontiguousarray(
                np.asarray(Wk, np.float32)[:, sl]).astype(bf16),
            "wv": np.ascontiguousarray(
                np.asarray(Wv, np.float32)[:, sl]).astype(bf16),
            "wg": np.ascontiguousarray(
                np.asarray(Wg, np.float32)[:, sl]).astype(bf16),
            "wo": np.ascontiguousarray(
                np.asarray(Wo, np.float32)[sl, :]).astype(bf16),
            "bq": np.ascontiguousarray(np.asarray(bq, np.float32)[sl]),
            "bk": np.ascontiguousarray(np.asarray(bk, np.float32)[sl]),
            "bv": np.ascontiguousarray(np.asarray(bv, np.float32)[sl]),
            "bg": np.ascontiguousarray(np.asarray(bg, np.float32)[sl]),
            "bo": (np.asarray(bo, np.float32) / group),
        })
    return in_maps


def _assemble(results, B=B, S=S, H=H, group=GROUP, nspl=None):
    if nspl is None:
        import os
        nspl = min(int(os.environ.get("KERNEL_NSPL", "4")), S // 512)
    rspl = min(2, nspl)
    Sn = S // rspl
    Sg = Sn // group
    out = np.empty((B, S, H), np.float32)
    for b in range(B):
        for j in range(group):
            y = results[b * group + j]["y_red"]
            for r in range(rspl):
                out[b, r * Sn + j * Sg: r * Sn + (j + 1) * Sg] = \
                    y[r * Sg:(r + 1) * Sg]
    return out


_NC_CACHE = {}


def get_program(with_bias=False):
    import os
    nspl = int(os.environ.get("KERNEL_NSPL", "4"))
    key = (B, S, H, D, nspl, with_bias)
    if key not in _NC_CACHE:
        _NC_CACHE[key] = build(nspl=nspl, with_bias=with_bias)
    return _NC_CACHE[key]


def kernel(hidden_states, Wq, bq, Wk, bk, Wv, bv, Wg, bg, Wo, bo):
    with_bias = any(
        np.any(np.asarray(b)) for b in (bq, bk, bv, bg, bo))
    nc = get_program(with_bias=with_bias)
    in_maps = _prep_inputs(hidden_states, Wq, bq, Wk, bk, Wv, bv, Wg, bg,
                           Wo, bo)
    res = run_bass_kernel_spmd(nc, in_maps, list(range(NCORE)))
    return _assemble(res.results)
